# revision 1
# baseline (speedup 1.0000x reference)
"""Trainium2 Bass kernel for a continuous bilinear Koopman operator rollout.

Problem (hardcoded shapes): z0 [256, 256] f32, kernel [256, 256] f32,
log_dt scalar, T=512.  Output: [256, 512, 256] f32 with
out[:, t, :] = z0 @ K_discrete^(t+1),
K_discrete = (I - 0.5*dt*K)^-1 (I + 0.5*dt*K), dt = exp(log_dt).

Strategy (v2):
  - Host (f64) computes K_discrete, its powers A^1..A^16, and the 32
    chunk-start states s_k = z0 @ A^(16k).  Everything ships as bf16.
  - z0/output sharded across 8 cores along batch (32 trajectories per
    core) -- pure data parallelism per the sharding hint.
  - Device does ONLY the output-producing matmuls ("phase C"):
    per group g (4 chunks x 32 batch = 128 partitions), 8 PSUM tiles
    [128, 512] accumulate s_k @ A^j over the two 128-halves of the
    contraction; DVE/ACT cast-copy f32 PSUM -> bf16 stage; one
    512 KB DMA per (group, queue-pair) drains 16-step chunks as
    8 KB-contiguous-per-partition packets (bigger packets = more
    per-queue DMA bandwidth; the 2 HWDGE queues are the bottleneck).
  - Output is written bf16 (half the HBM write traffic) and upcast to
    f32 on the host.  Total rel-err ~2e-3 vs the 2e-2 gate.
"""

import numpy as np

B = 256
D = 256
T = 512
N_CORES = 8
B_LOC = B // N_CORES      # 32
C = 16                    # chunk length (powers A^1..A^C shipped)
N_CHUNKS = T // C         # 32
N_GROUPS = N_CHUNKS // 4  # 8 groups of 4 chunks -> M=128
JP = C // 2               # 8 pairs of consecutive powers -> N=512

# qin column layout (bf16, [128, QCOLS]):
#   [0:1024)          S[h=0]: [r, k*32+b] = s_k[b, r]
#   [1024:2048)       S[h=1]: [r, k*32+b] = s_k[b, 128+r]
#   [2048 + pp*2048 + h*1024 + u*256 + c] = A^(4*pp+1+u)[h*128+r, c]
#   (pp = quads of consecutive powers -> N=1024 fused matmuls)
S_COLS = 2 * N_CHUNKS * B_LOC          # 2048
QCOLS = S_COLS + JP * 1024             # 10240
K8 = 20                                # chunks 0..K8-1 stored as scaled fp8
G8 = K8 // 4                           # 5 fp8 groups (of 8)

_CACHE = {}


def _build_bass(fp8_scale):
    import concourse.tile as tile
    from concourse import bacc, mybir

    f32 = mybir.dt.float32
    bf16 = mybir.dt.bfloat16
    fp8 = mybir.dt.float8e4
    nc = bacc.Bacc("TRN2", target_bir_lowering=False, debug=False)

    qin = nc.dram_tensor("qin", [128, QCOLS], bf16, kind="ExternalInput").ap()
    # Chunk-major outputs: row (k*32 + b) holds chunk k of trajectory b
    # (16 steps x 256 dims).  Early chunks (k < K8, ~4% of norm energy)
    # are written as scaled fp8e4 -- half the bytes again; late chunks
    # as bf16.  The host decodes and un-permutes to [B_LOC, T, D].
    out8 = nc.dram_tensor(
        "out8", [K8 * B_LOC, C * D], fp8, kind="ExternalOutput"
    ).ap()
    out16 = nc.dram_tensor(
        "out16", [(N_CHUNKS - K8) * B_LOC, C * D], bf16, kind="ExternalOutput"
    ).ap()
    # descriptor dim m (16 per drain), 4 partitions each
    out8_m = out8.rearrange("(m b) d -> m b d", b=4)
    out16_m = out16.rearrange("(m b) d -> m b d", b=4)

    with tile.TileContext(nc) as tc:
        with (
            tc.tile_pool(name="const", bufs=1) as cpool,
            tc.tile_pool(name="psum", bufs=4, space="PSUM") as psum_pool,
            tc.tile_pool(name="stage", bufs=8) as stage_pool,
        ):
            Q = cpool.tile([128, QCOLS], bf16, name="q")

            # ---- input DMAs: S+pp0 first (unblocks the PE early), pp1-2
            # on scalar, pp3 on sync ----
            nc.sync.dma_start(Q[:, 0:4096], qin[:, 0:4096])
            nc.scalar.dma_start(Q[:, 4096:8192], qin[:, 4096:8192])
            nc.sync.dma_start(Q[:, 8192:QCOLS], qin[:, 8192:QCOLS])

            def s_slice(h, g):
                return Q[:, h * 1024 + g * 128: h * 1024 + (g + 1) * 128]

            def p_slice(h, pp):
                base = S_COLS + pp * 2048 + h * 1024
                return Q[:, base: base + 1024]

            def group(g):
                is8 = g < G8
                stage = stage_pool.tile(
                    [128, C * D], fp8 if is8 else bf16, name="stage"
                )
                # 4 powers per 2-bank PSUM tile (N=512 matmuls -- ISA cap);
                # one 1024-wide copy per tile, alternating DVE/ACT.
                for pp in range(4):
                    ps = psum_pool.tile([128, 1024], f32, name="ps", tag="ps")
                    for q in range(2):
                        for h in range(2):
                            nc.tensor.matmul(
                                ps[:, q * 512:(q + 1) * 512],
                                s_slice(h, g),
                                p_slice(h, pp)[:, q * 512:(q + 1) * 512],
                                start=(h == 0),
                                stop=(h == 1),
                            )
                    dst = stage[:, pp * 1024:(pp + 1) * 1024]
                    if pp % 2 == 0:
                        if is8:
                            nc.vector.tensor_scalar_mul(dst, ps[:], fp8_scale)
                        else:
                            nc.vector.tensor_copy(dst, ps[:])
                    else:
                        if is8:
                            nc.scalar.mul(dst, ps[:], fp8_scale)
                        else:
                            nc.scalar.copy(dst, ps[:])
                # Drain: 2 DMAs per group (2 chunks each).  AP [16, 4, 4096]:
                # 16 descriptors spread across the DMA engines.
                out_m = out8_m if is8 else out16_m
                gl = g if is8 else g - G8
                for qi in range(2):
                    dma_eng = nc.sync if qi == 0 else nc.scalar
                    m0 = (4 * gl + 2 * qi) * 8
                    dma_eng.dma_start(
                        out_m[m0: m0 + 16, :, :],
                        stage[qi * 64:(qi + 1) * 64, :],
                    )

            # bf16 groups first; fp8 groups last (smaller tail drains)
            for g in list(range(G8, N_GROUPS)) + list(range(G8)):
                group(g)

    nc.compile()
    return nc


def _host_prep(z0, kernel, log_dt):
    """fp64 host math: K_discrete, powers A^1..A^16, chunk starts."""
    K = np.asarray(kernel, dtype=np.float64)
    dt = float(np.exp(np.float64(np.asarray(log_dt))))
    eye = np.eye(D, dtype=np.float64)
    A = np.linalg.solve(eye - 0.5 * dt * K, eye + 0.5 * dt * K)

    pows = [None] * (C + 1)  # pows[j] = A^j
    pows[1] = A
    for j in range(2, C + 1):
        pows[j] = pows[j - 1] @ A

    # chunk starts: s_k = z0 @ A^(16k), k = 0..31   [32, B, D]
    z = np.asarray(z0, dtype=np.float64)
    s_list = [z]
    for _ in range(N_CHUNKS - 1):
        s_list.append(s_list[-1] @ pows[C])
    s_all = np.stack(s_list, axis=0)  # [32, B, D]

    import ml_dtypes

    bf16 = ml_dtypes.bfloat16

    # P tail [128, 4*2048]: [r, pp*2048 + h*1024 + u*256 + c]
    #   = A^(4pp+1+u)[h*128+r, c]
    parr = np.stack([pows[j] for j in range(1, C + 1)], axis=0)  # [16, 256, 256]
    ptail = np.ascontiguousarray(
        parr.reshape(4, 4, 2, 128, D)         # [pp, u, h, r, c]
        .transpose(3, 0, 2, 1, 4)             # [r, pp, h, u, c]
        .reshape(128, 4 * 2048)
    ).astype(bf16)

    # Per-core S block [128, 2048]: [r, h*1024 + k*32 + b] = s_k[b, h*128+r]
    qins = []
    for cidx in range(N_CORES):
        sc = s_all[:, cidx * B_LOC:(cidx + 1) * B_LOC, :]   # [k, b, 256]
        sblk = np.ascontiguousarray(
            sc.reshape(N_CHUNKS, B_LOC, 2, 128)   # [k, b, h, r]
            .transpose(3, 2, 0, 1)                # [r, h, k, b]
            .reshape(128, S_COLS)
        ).astype(bf16)
        qins.append(np.ascontiguousarray(np.concatenate([sblk, ptail], axis=1)))

    # fp8 scale: rigorous bound |s_k @ A^j| <= max_row ||s_k|| * max_col ||A^j||
    # over the fp8 chunks (k < K8), rounded to a power of two under 240.
    rownorm = max(
        float(np.linalg.norm(s_all[k], axis=1).max()) for k in range(K8)
    )
    colnorm = max(
        float(np.linalg.norm(pows[j], axis=0).max()) for j in range(1, C + 1)
    )
    bound = rownorm * colnorm * 1.05
    fp8_scale = float(2.0 ** np.floor(np.log2(240.0 / bound)))
    return qins, fp8_scale


def kernel(**inputs):
    from concourse.bass_utils import run_bass_kernel_spmd

    z0 = inputs["z0"]
    kmat = inputs["kernel"]
    log_dt = inputs["log_dt"]
    t_in = int(np.asarray(inputs["T"]))
    assert t_in == T, f"kernel hardcoded for T={T}, got {t_in}"
    assert tuple(np.shape(z0)) == (B, D)

    qins, fp8_scale = _host_prep(z0, kmat, log_dt)

    if _CACHE.get("scale") != fp8_scale:
        _CACHE["nc"] = _build_bass(fp8_scale)
        _CACHE["scale"] = fp8_scale
    nc = _CACHE["nc"]

    in_maps = [{"qin": qins[c]} for c in range(N_CORES)]
    res = run_bass_kernel_spmd(nc, in_maps, core_ids=list(range(N_CORES)))
    outs = []
    for c in range(N_CORES):
        o8 = np.asarray(res.results[c]["out8"]).astype(np.float32) / fp8_scale
        o16 = np.asarray(res.results[c]["out16"]).astype(np.float32)
        o = np.concatenate([o8, o16], axis=0)  # [1024, 4096] chunk-major
        o = (
            o.reshape(N_CHUNKS, B_LOC, C, D)
            .transpose(1, 0, 2, 3)
            .reshape(B_LOC, T, D)
        )
        outs.append(o)
    return np.concatenate(outs, axis=0)



# revision 2
# speedup vs baseline: 1.3454x; 1.3454x over previous
"""Trainium2 Bass kernel for a continuous bilinear Koopman operator rollout.

Problem (hardcoded shapes): z0 [256, 256] f32, kernel [256, 256] f32,
log_dt scalar, T=512.  Output: [256, 512, 256] f32 with
out[:, t, :] = z0 @ K_discrete^(t+1),
K_discrete = (I - 0.5*dt*K)^-1 (I + 0.5*dt*K), dt = exp(log_dt).

Strategy (v3):
  - Host (f64) computes K_discrete, powers A^1..A^16, and the 32
    chunk-start states s_k = z0 @ A^(16k).  z0/output sharded across 8
    cores along batch (32 trajectories per core).
  - Device computes out-chunk rows s_k @ A^j (j=1..16) as matmuls:
    per group (4 chunks x 32 batch = 128 partitions), PSUM tiles
    [128, 1024] accumulate over the contraction r=256.
  - Early chunks (0..19, ~4%% of output energy) are computed entirely
    in fp8 (e4m3) with DoubleRow perf mode: K=256 in one matmul at 2x
    PE throughput.  Chunks 20..31 are computed in bf16.
  - Chunks 0..23 are written to HBM as scaled fp8 (1 B/elem); chunks
    24..31 as bf16.  Host decodes.  Predicted rel-err ~9.7e-3 vs the
    2e-2 gate (validated with an exact host-side quantization sim).
  - Input DMAs are split in consumption order across both HWDGE
    queues so the first matmul starts ~1us after the preamble ends.
  - Output drains are full 128-partition DMAs with 8-16KB contiguous
    per-partition descriptors, rotated across sync(HWDGE) and
    gpsimd(SWDGE) queues so they overlap compute and each engage all
    16 SDMA engines.
"""

import numpy as np

B = 256
D = 256
T = 512
N_CORES = 8
B_LOC = B // N_CORES      # 32
C = 16                    # chunk length (powers A^1..A^C shipped)
N_CHUNKS = T // C         # 32
N_GROUPS = N_CHUNKS // 4  # 8 groups of 4 chunks -> M=128

# group processing order: bf16-computed groups first (their operands
# arrive first), fp8 groups last (smaller drain tail).
GORDER = [6, 7, 5, 0, 1, 2, 3, 4]
G16C = [6, 7, 5]           # bf16-computed groups (chunks 20..31)
G8C = [0, 1, 2, 3, 4]      # fp8-computed groups (chunks 0..19)
FP8W = {0, 1, 2, 3, 4, 5}  # fp8-written groups (chunks 0..23)

# qin16 [128, 8960] bf16:
#   [0:768)    S16 blocks, one 256-col block per gi in (6,7,5):
#              col gi*256 + h*128 + (a*32+b) = s_{4g+a}[cb+b, h*128+r]
#   [768:8960) P16: col 768 + pp*2048 + q*1024 + h*512 + u2*256 + d
#              = A^{4pp+1+2q+u2}[h*128+r, d]
S16_COLS = len(G16C) * 256          # 768
P16_BASE = S16_COLS
Q16_COLS = S16_COLS + 8192          # 8960
# qin8 [128, 9472] fp8, viewed [128, 2(h), 4736]:
#   within h: [0:640) S8 blocks per g in G8C (scaled by s8sc);
#             [640:4736) P8 same (pp,q,u2,d) order as P16 (scaled p8sc)
Q8_S = len(G8C) * 128               # 640
Q8_H = Q8_S + 4096                  # 4736

# stage-half assignment: (stage_tensor_index, column half)
#   st16: g6 half0, g7 half1 (bf16)
#   st8a: g5 half0, g0 half1; st8b: g1, g2; st8c: g3, g4 (fp8)
_CACHE = {}


def _build_bass(eff8, f8s):
    import concourse.tile as tile
    from concourse import bacc, mybir

    f32 = mybir.dt.float32
    bf16 = mybir.dt.bfloat16
    fp8 = mybir.dt.float8e4
    DR = mybir.MatmulPerfMode.DoubleRow
    nc = bacc.Bacc("TRN2", target_bir_lowering=False, debug=False)

    qin16 = nc.dram_tensor("qin16", [128, Q16_COLS], bf16, kind="ExternalInput").ap()
    qin8 = nc.dram_tensor("qin8", [128, 2 * Q8_H], fp8, kind="ExternalInput").ap()
    out16 = nc.dram_tensor("out16", [128, 8192], bf16, kind="ExternalOutput").ap()
    out8 = nc.dram_tensor("out8", [128, 24576], fp8, kind="ExternalOutput").ap()
    qin8v = qin8.rearrange("p (h c) -> p h c", h=2)

    with tile.TileContext(nc) as tc:
        with (
            tc.tile_pool(name="const", bufs=1) as cpool,
            tc.tile_pool(name="psum", bufs=4, space="PSUM") as psum_pool,
        ):
            Q16 = cpool.tile([128, Q16_COLS], bf16, name="q16")
            Q8 = cpool.tile([128, 2, Q8_H], fp8, name="q8")
            st16 = cpool.tile([128, 8192], bf16, name="st16")
            st8a = cpool.tile([128, 8192], fp8, name="st8a")
            st8b = cpool.tile([128, 8192], fp8, name="st8b")
            st8c = cpool.tile([128, 8192], fp8, name="st8c")
            sthalf = {6: (st16, 0), 7: (st16, 1), 5: (st8a, 0), 0: (st8a, 1),
                      1: (st8b, 0), 2: (st8b, 1), 3: (st8c, 0), 4: (st8c, 1)}

            # ---- input DMAs in consumption order, both HWDGE queues ----
            nc.sync.dma_start(Q16[:, 0:256], qin16[:, 0:256])          # S16 g6
            nc.scalar.dma_start(Q16[:, 768:1792], qin16[:, 768:1792])  # pp0 q0
            nc.sync.dma_start(Q16[:, 256:768], qin16[:, 256:768])      # S16 g7,g5
            nc.scalar.dma_start(Q16[:, 1792:2816], qin16[:, 1792:2816])  # pp0 q1
            nc.sync.dma_start(Q16[:, 2816:4864], qin16[:, 2816:4864])  # pp1
            nc.scalar.dma_start(Q16[:, 4864:6912], qin16[:, 4864:6912])  # pp2
            nc.sync.dma_start(Q16[:, 6912:8960], qin16[:, 6912:8960])  # pp3
            nc.scalar.dma_start(Q8[:, 0:1, :], qin8v[:, 0:1, :])
            nc.sync.dma_start(Q8[:, 1:2, :], qin8v[:, 1:2, :])

            def do_group(gi):
                g = GORDER[gi]
                stt, half = sthalf[g]
                for pp in range(4):
                    ps = psum_pool.tile([128, 1024], f32, name="ps", tag="ps")
                    if g in G8C:
                        si = G8C.index(g)
                        for q in range(2):
                            pb = Q8_S + pp * 1024 + q * 512
                            nc.tensor.matmul(
                                ps[:, q * 512:(q + 1) * 512],
                                Q8[:, :, si * 128:(si + 1) * 128],
                                Q8[:, :, pb:pb + 512],
                                start=True, stop=True, perf_mode=DR,
                            )
                    else:
                        for q in range(2):
                            for h in range(2):
                                pb = P16_BASE + pp * 2048 + q * 1024 + h * 512
                                nc.tensor.matmul(
                                    ps[:, q * 512:(q + 1) * 512],
                                    Q16[:, gi * 256 + h * 128: gi * 256 + (h + 1) * 128],
                                    Q16[:, pb:pb + 512],
                                    start=(h == 0), stop=(h == 1),
                                )
                    dst = stt[:, half * 4096 + pp * 1024: half * 4096 + (pp + 1) * 1024]
                    if g in FP8W:
                        sc = eff8 if g in G8C else f8s
                        if pp % 2 == 0:
                            nc.vector.tensor_scalar_mul(dst, ps[:], sc)
                        else:
                            nc.scalar.mul(dst, ps[:], sc)
                    else:
                        if pp % 2 == 0:
                            nc.vector.tensor_copy(dst, ps[:])
                        else:
                            nc.scalar.copy(dst, ps[:])
                # drains: full 128-partition DMAs, 8-16KB per partition
                if g == 6:
                    nc.sync.dma_start(out16[:, 0:4096], st16[:, 0:4096])
                elif g == 7:
                    nc.gpsimd.dma_start(out16[:, 4096:8192], st16[:, 4096:8192])
                elif g == 0:   # g5 finished earlier; pair drain (5, 0)
                    nc.sync.dma_start(out8[:, 0:8192], st8a[:, :])
                elif g == 2:   # pair drain (1, 2)
                    nc.gpsimd.dma_start(out8[:, 8192:16384], st8b[:, :])
                elif g == 3:   # tail pair split for earlier issue
                    nc.sync.dma_start(out8[:, 16384:20480], st8c[:, 0:4096])
                elif g == 4:
                    nc.gpsimd.dma_start(out8[:, 20480:24576], st8c[:, 4096:8192])

            for gi in range(N_GROUPS):
                do_group(gi)

    nc.compile()
    return nc


def _pow2floor(x):
    return float(2.0 ** np.floor(np.log2(x)))


def _host_prep(z0, kernel, log_dt):
    """fp64 host math: K_discrete, powers, chunk starts; pack qin16/qin8."""
    import ml_dtypes

    BF16 = ml_dtypes.bfloat16
    FP8NP = ml_dtypes.float8_e4m3

    K = np.asarray(kernel, dtype=np.float64)
    dt = float(np.exp(np.float64(np.asarray(log_dt))))
    eye = np.eye(D, dtype=np.float64)
    A = np.linalg.solve(eye - 0.5 * dt * K, eye + 0.5 * dt * K)

    pows = [None] * (C + 1)
    pows[1] = A
    for j in range(2, C + 1):
        pows[j] = pows[j - 1] @ A

    z = np.asarray(z0, dtype=np.float64)
    s_list = [z]
    for _ in range(N_CHUNKS - 1):
        s_list.append(s_list[-1] @ pows[C])
    s_all = np.stack(s_list, axis=0)  # [32, B, D]

    # scales (powers of two)
    s8max = max(float(np.abs(s_all[4 * g: 4 * g + 4]).max()) for g in G8C)
    s8sc = _pow2floor(240.0 / (1.05 * s8max))
    p8max = max(float(np.abs(pows[j]).max()) for j in range(1, C + 1))
    p8sc = _pow2floor(240.0 / (1.05 * p8max))
    n_fp8w_chunks = 4 * len(FP8W)
    rownorm = max(
        float(np.linalg.norm(s_all[k], axis=1).max()) for k in range(n_fp8w_chunks)
    )
    colnorm = max(
        float(np.linalg.norm(pows[j], axis=0).max()) for j in range(1, C + 1)
    )
    f8s = _pow2floor(240.0 / (1.05 * rownorm * colnorm))
    eff8 = f8s / (s8sc * p8sc)

    # P16 region [128, 8192]
    p16 = np.empty((128, 8192), dtype=np.float64)
    for pp in range(4):
        for q in range(2):
            for h in range(2):
                for u2 in range(2):
                    j = 4 * pp + 1 + 2 * q + u2
                    base = pp * 2048 + q * 1024 + h * 512 + u2 * 256
                    p16[:, base:base + 256] = pows[j][h * 128:(h + 1) * 128, :]

    # P8 per-h region [2][128, 4096]
    p8h = np.empty((2, 128, 4096), dtype=np.float64)
    for h in range(2):
        for pp in range(4):
            for q in range(2):
                for u2 in range(2):
                    j = 4 * pp + 1 + 2 * q + u2
                    col = pp * 1024 + q * 512 + u2 * 256
                    p8h[h][:, col:col + 256] = (
                        pows[j][h * 128:(h + 1) * 128, :] * p8sc
                    )

    def s_block(g, cb, h):
        # [128 r, 128 (a*32+b)] = s_{4g+a}[cb+b, h*128+r]
        blk = s_all[4 * g: 4 * g + 4, cb:cb + 32, h * 128:(h + 1) * 128]
        return blk.transpose(2, 0, 1).reshape(128, 128)

    in_maps = []
    for c in range(N_CORES):
        cb = c * B_LOC
        q16 = np.empty((128, Q16_COLS), dtype=np.float64)
        for gi, g in enumerate(G16C):
            for h in range(2):
                q16[:, gi * 256 + h * 128: gi * 256 + (h + 1) * 128] = s_block(g, cb, h)
        q16[:, P16_BASE:] = p16
        q8 = np.empty((128, 2 * Q8_H), dtype=np.float64)
        for h in range(2):
            off = h * Q8_H
            for si, g in enumerate(G8C):
                q8[:, off + si * 128: off + (si + 1) * 128] = s_block(g, cb, h) * s8sc
            q8[:, off + Q8_S: off + Q8_H] = p8h[h]
        in_maps.append({
            "qin16": np.ascontiguousarray(q16).astype(BF16),
            "qin8": np.ascontiguousarray(q8).astype(FP8NP),
        })
    scales = {"fp8_scale": f8s, "eff8": eff8, "s8sc": s8sc, "p8sc": p8sc}
    return in_maps, scales


def _decode(res_c, f8s):
    """Device outputs [out16 [128,8192] bf16, out8 [128,24576] fp8]
    -> [B_LOC, T, D] f32."""
    o16 = np.asarray(res_c["out16"]).astype(np.float32)
    o8 = np.asarray(res_c["out8"]).astype(np.float32) / f8s
    out = np.empty((B_LOC, T, D), dtype=np.float32)

    def put(arr, g):  # arr [128, 4096]: [a*32+b, t_local*256+d]
        out[:, 4 * g * C:(4 * g + 4) * C, :] = (
            arr.reshape(4, 32, C, D).transpose(1, 0, 2, 3).reshape(32, 4 * C, D)
        )

    put(o16[:, 0:4096], 6)
    put(o16[:, 4096:8192], 7)
    for ri, (ga, gb) in enumerate([(5, 0), (1, 2), (3, 4)]):
        put(o8[:, ri * 8192: ri * 8192 + 4096], ga)
        put(o8[:, ri * 8192 + 4096: (ri + 1) * 8192], gb)
    return out


def kernel(**inputs):
    from concourse.bass_utils import run_bass_kernel_spmd

    z0 = inputs["z0"]
    kmat = inputs["kernel"]
    log_dt = inputs["log_dt"]
    t_in = int(np.asarray(inputs["T"]))
    assert t_in == T, f"kernel hardcoded for T={T}, got {t_in}"
    assert tuple(np.shape(z0)) == (B, D)

    in_maps, scales = _host_prep(z0, kmat, log_dt)

    key = (scales["fp8_scale"], scales["eff8"])
    if _CACHE.get("key") != key:
        _CACHE["nc"] = _build_bass(scales["eff8"], scales["fp8_scale"])
        _CACHE["key"] = key
    nc = _CACHE["nc"]

    res = run_bass_kernel_spmd(nc, in_maps, core_ids=list(range(N_CORES)))
    outs = [_decode(res.results[c], scales["fp8_scale"]) for c in range(N_CORES)]
    return np.concatenate(outs, axis=0)
